# revision 1
# baseline (speedup 1.0000x reference)
"""Trainium2 Bass kernel for OPT attention with H2O heavy-hitter masking.

Distribution: batch*heads (BH=32) sharded across 8 NeuronCores, 4 heads each
(head-parallel). Each core computes QKV projections for its heads, attention
scores, the sequential heavy-hitter eviction scan, the masked softmax + @V,
and a tensor-parallel slice of the output projection; the host sums the
per-core partials (the all-reduce) and adds the output bias.

Key device-side layouts (S=2048 sequence, NH=4 heads/core, D=64):
  - scan layout for one row t: tile [128, FSC], FSC=S/32.
    partition p = 32*h + (j // FSC), free f = j % FSC  (j = key index),
    i.e. flat [4, S] per row, contiguous in DRAM.
  - awS   DRAM [S, 4*S] f32: causal-masked scores, row-major per head.
  - masksS DRAM [S, 4*S] f32: recorded heavy-hitter masks (0/1).
  - qT/kT SBUF [2][128, S]: transposed projections (head dims on partitions).
  - v SBUF [128, S/128, 256]; attnT SBUF [2][128, S].

The scan keeps a scaled accumulator B = acc * s_t (s_t = current softmax
denominator), so each step is: mask -> exp -> masked sum -> ratio
r = p*s_t/s_{t-1} -> B = B*r + e -> masked min (via block transpose-reduce)
-> match_replace eviction.  argmin/top-k order is preserved because B is a
positive rescaling of acc.
"""

import numpy as np
from contextlib import ExitStack

S = 2048
E = 1024
H = 16
NH = 4   # heads per core
D = 64
PEN = 0.98
HB = int(0.1 * S)
RB = int(0.1 * S)
SCALE = D ** -0.5
BIG = 1e30

_CACHE = {}


def _build(with_bias, maskmin):
    import concourse.tile as tile
    from concourse import bacc, mybir

    f32 = mybir.dt.float32
    Alu = mybir.AluOpType
    Act = mybir.ActivationFunctionType
    X = mybir.AxisListType.X

    hb, rb = HB, RB
    FSC = S // 32          # scan-layout free size
    NQ = S // 128          # q-tiles
    KC = E // 128          # contraction chunks for projections
    NW = min(512, S)       # matmul N chunk
    emin = float(np.exp(np.float64(maskmin))) if maskmin > -80 else 0.0

    nc = bacc.Bacc("TRN2", target_bir_lowering=False, debug=False, num_devices=8)

    hsT = nc.dram_tensor("hsT", [E, S], f32, kind="ExternalInput").ap()
    wqT = nc.dram_tensor("wqT", [E, 256], f32, kind="ExternalInput").ap()
    wkT = nc.dram_tensor("wkT", [E, 256], f32, kind="ExternalInput").ap()
    wvT = nc.dram_tensor("wvT", [E, 256], f32, kind="ExternalInput").ap()
    woT = nc.dram_tensor("woT", [256, E], f32, kind="ExternalInput").ap()
    amask = nc.dram_tensor("amask", [S, S], f32, kind="ExternalInput").ap()
    oneh = nc.dram_tensor("oneh", [S, 128 * FSC], f32, kind="ExternalInput").ap()
    band = nc.dram_tensor("band", [128, 2 * S], f32, kind="ExternalInput").ap()
    ident = nc.dram_tensor("ident", [128, 128], f32, kind="ExternalInput").ap()
    idxf = nc.dram_tensor("idxf", [128, FSC], f32, kind="ExternalInput").ap()
    if with_bias:
        bq2 = nc.dram_tensor("bq2", [128, 2], f32, kind="ExternalInput").ap()
        bk2 = nc.dram_tensor("bk2", [128, 2], f32, kind="ExternalInput").ap()
        bvr = nc.dram_tensor("bvr", [128, 256], f32, kind="ExternalInput").ap()
    partial = nc.dram_tensor("partial", [S, E], f32, kind="ExternalOutput").ap()

    awS = nc.dram_tensor("awS", [S, NH * S], f32).ap()
    masksS = nc.dram_tensor("masksS", [S, NH * S], f32).ap()

    with tile.TileContext(nc) as tc, ExitStack() as top:
        cpool = top.enter_context(tc.tile_pool(name="consts", bufs=1))
        ppool = top.enter_context(tc.tile_pool(name="persist", bufs=1))

        idxf_sb = cpool.tile([128, FSC], f32)
        nc.sync.dma_start(idxf_sb[:], idxf)
        band_sb = cpool.tile([128, 2 * S], f32)
        nc.sync.dma_start(band_sb[:], band)
        ident_sb = cpool.tile([128, 128], f32)
        nc.sync.dma_start(ident_sb[:], ident)
        if with_bias:
            bq_sb = cpool.tile([128, 2], f32)
            nc.sync.dma_start(bq_sb[:], bq2)
            bk_sb = cpool.tile([128, 2], f32)
            nc.sync.dma_start(bk_sb[:], bk2)
            bv_sb = cpool.tile([128, 256], f32)
            nc.sync.dma_start(bv_sb[:], bvr)

        qT_sb = [ppool.tile([128, S], f32, tag=f"qT{m}", name=f"qT{m}")
                 for m in range(2)]
        kT_sb = [ppool.tile([128, S], f32, tag=f"kT{m}", name=f"kT{m}")
                 for m in range(2)]
        v_sb = ppool.tile([128, S // 128, 256], f32, tag="v")

        # ============ PHASE 1a: projections ============
        with tc.tile_pool(name="ph1", bufs=5) as hsp, \
             tc.tile_pool(name="ph1w", bufs=1) as wpool, \
             tc.tile_pool(name="ph1ps", bufs=1, space="PSUM") as ps1:
            w_sb = []
            for wi in range(3):
                wt = (wqT, wkT, wvT)[wi]
                wsb = wpool.tile([128, KC, 256], f32, name=f"w_sb{wi}",
                                 tag=f"w_sb{wi}")
                nc.sync.dma_start(wsb[:], wt.rearrange("(kc p) n -> p kc n", p=128))
                w_sb.append(wsb)
            for proj in range(3):  # 0=q 1=k 2=v
                wT = w_sb[proj]
                if proj < 2:
                    psums = []
                    for mc in range(2):
                        for nb in range(S // NW):
                            psums.append(ps1.tile(
                                [128, NW], f32, name=f"ps_{mc}_{nb}",
                                tag=f"ps{(mc * (S // NW) + nb) % 8}"))
                    for kc in range(KC):
                        ht = hsp.tile([128, S], f32, tag="hsT")
                        nc.sync.dma_start(ht[:], hsT[kc * 128:(kc + 1) * 128, :])
                        for mc in range(2):
                            for nb in range(S // NW):
                                nc.tensor.matmul(
                                    psums[mc * (S // NW) + nb][:],
                                    wT[:, kc, mc * 128:(mc + 1) * 128],
                                    ht[:, nb * NW:(nb + 1) * NW],
                                    start=(kc == 0), stop=(kc == KC - 1))
                    dst = qT_sb if proj == 0 else kT_sb
                    sc = SCALE if proj == 0 else 1.0
                    b_sb = (bq_sb if proj == 0 else bk_sb) if with_bias else None
                    for mc in range(2):
                        for nb in range(S // NW):
                            ps = psums[mc * (S // NW) + nb]
                            o = dst[mc][:, nb * NW:(nb + 1) * NW]
                            if with_bias:
                                nc.vector.tensor_scalar(
                                    o, ps[:], b_sb[:, mc:mc + 1], sc,
                                    op0=Alu.add, op1=Alu.mult)
                            elif sc != 1.0:
                                nc.scalar.mul(o, ps[:], sc)
                            else:
                                nc.scalar.copy(o, ps[:])
                else:
                    for w0 in range(0, S // 128, 8):
                        wn = min(8, S // 128 - w0)
                        psums = [ps1.tile([128, 256], f32,
                                          name=f"psv_{w0 + i}", tag=f"ps{i}")
                                 for i in range(wn)]
                        for kc in range(KC):
                            ht = hsp.tile([128, S], f32, tag="hsT")
                            nc.sync.dma_start(ht[:],
                                              hsT[kc * 128:(kc + 1) * 128, :])
                            for i in range(wn):
                                tc_ = w0 + i
                                nc.tensor.matmul(
                                    psums[i][:],
                                    ht[:, tc_ * 128:(tc_ + 1) * 128],
                                    wT[:, kc, :],
                                    start=(kc == 0), stop=(kc == KC - 1))
                        for i in range(wn):
                            o = v_sb[:, w0 + i, :]
                            if with_bias:
                                nc.vector.tensor_add(o, psums[i][:], bv_sb[:])
                            else:
                                nc.scalar.copy(o, psums[i][:])

        # ============ PHASE 1b: attention scores ============
        with tc.tile_pool(name="ph1b", bufs=2) as mp, \
             tc.tile_pool(name="ph1bo", bufs=3) as op, \
             tc.tile_pool(name="ph1bps", bufs=2, space="PSUM") as ps2:
            for qb in range(NQ):
                am_t = mp.tile([128, S], f32, tag="amask")
                nc.sync.dma_start(am_t[:], amask[qb * 128:(qb + 1) * 128, :])
                for h in range(NH):
                    mc, mo = h // 2, 64 * (h % 2)
                    awsb = op.tile([128, S], f32, tag="awsb")
                    for kb in range(S // NW):
                        aps = ps2.tile([128, NW], f32, tag=f"awps{kb % 4}")
                        nc.tensor.matmul(
                            aps[:],
                            qT_sb[mc][mo:mo + 64, qb * 128:(qb + 1) * 128],
                            kT_sb[mc][mo:mo + 64, kb * NW:(kb + 1) * NW],
                            start=True, stop=True)
                        nc.vector.scalar_tensor_tensor(
                            awsb[:, kb * NW:(kb + 1) * NW], aps[:], 1.0,
                            am_t[:, kb * NW:(kb + 1) * NW],
                            op0=Alu.mult, op1=Alu.add)
                    nc.sync.dma_start(
                        awS[qb * 128:(qb + 1) * 128, h * S:(h + 1) * S], awsb[:])

        # ============ PHASE 2: heavy-hitter scan ============
        B_t = ppool.tile([128, FSC], f32, tag="B")
        vt8 = ppool.tile([128, 8], f32, tag="vt8")
        nc.vector.memset(B_t[:], 0.0)
        nc.vector.memset(vt8[:], -1.0)

        with tc.tile_pool(name="scrow", bufs=8) as rowp, \
             tc.tile_pool(name="sceraw", bufs=4) as erp, \
             tc.tile_pool(name="scmf", bufs=4) as mfp, \
             tc.tile_pool(name="scsmall", bufs=4) as smp, \
             tc.tile_pool(name="scscr", bufs=4) as scrp, \
             tc.tile_pool(name="scinv", bufs=2) as invp:

            invprev = invp.tile([128, 1], f32, tag="inv")
            nc.vector.memset(invprev[:], 1.0)

            # ---- prologue rows 0..hb-1 ----
            for t in range(hb):
                rowt = rowp.tile([128, FSC], f32, tag="row")
                nc.sync.dma_start(rowt[:],
                                  awS[t, :].rearrange("(p f) -> p f", p=128))
                part = smp.tile([128, 1], f32, tag="part")
                eraw = erp.tile([128, FSC], f32, tag="eraw")
                nc.scalar.activation(eraw[:], rowt[:], Act.Exp, accum_out=part[:])
                bc = scrp.tile([128, 32], f32, tag="bc")
                nc.vector.tensor_copy(bc[:], part[:].to_broadcast([128, 32]))
                s_t = smp.tile([128, 1], f32, tag="s")
                nc.vector.tensor_reduce(s_t[:], bc[:], axis=X, op=Alu.add,
                                        apply_transpose=True)
                w_t = smp.tile([128, 1], f32, tag="w")
                nc.vector.reciprocal(w_t[:], s_t[:])
                pf = PEN ** (hb - 1 - t)
                wt2 = smp.tile([128, 1], f32, tag="w2")
                nc.vector.tensor_scalar(wt2[:], w_t[:], pf, None, op0=Alu.mult)
                nc.vector.scalar_tensor_tensor(
                    B_t[:], eraw[:], wt2[:], B_t[:], op0=Alu.mult, op1=Alu.add)
                mrow = mfp.tile([128, FSC], f32, tag="mf")
                nc.vector.tensor_scalar(mrow[:], idxf_sb[:], float(t), None,
                                        op0=Alu.is_le)
                nc.sync.dma_start(
                    masksS[t, :].rearrange("(p f) -> p f", p=128), mrow[:])
                if t == hb - 1:
                    nc.vector.tensor_mul(B_t[:], B_t[:], mrow[:])

            # ---- main scan ----
            for t in range(hb, S + 1):
                cmpt = rowp.tile([128, FSC], f32, tag="cmp")
                nc.sync.dma_start(
                    cmpt[:], oneh[t - 1, :].rearrange("(p f) -> p f", p=128))
                mf = mfp.tile([128, FSC], f32, tag="mf")
                nc.vector.scalar_tensor_tensor(
                    mf[:], B_t[:], 0.0, cmpt[:], op0=Alu.is_gt, op1=Alu.max)
                nc.sync.dma_start(
                    masksS[t - 1, :].rearrange("(p f) -> p f", p=128), mf[:])
                if t == S:
                    break
                rowt = rowp.tile([128, FSC], f32, tag="row")
                nc.sync.dma_start(rowt[:],
                                  awS[t, :].rearrange("(p f) -> p f", p=128))
                eraw = erp.tile([128, FSC], f32, tag="eraw")
                nc.scalar.activation(eraw[:], rowt[:], Act.Exp)
                e_t = erp.tile([128, FSC], f32, tag="e")
                part = smp.tile([128, 1], f32, tag="part")
                nc.vector.scalar_tensor_tensor(
                    e_t[:], eraw[:], 1.0, mf[:], op0=Alu.mult, op1=Alu.mult,
                    accum_out=part[:])
                bc = scrp.tile([128, 32], f32, tag="bc")
                nc.vector.tensor_copy(bc[:], part[:].to_broadcast([128, 32]))
                s_t = smp.tile([128, 1], f32, tag="s")
                nc.vector.tensor_reduce(s_t[:], bc[:], axis=X, op=Alu.add,
                                        apply_transpose=True)
                r_t = smp.tile([128, 1], f32, tag="r")
                nc.vector.tensor_scalar(r_t[:], s_t[:], invprev[:], PEN,
                                        op0=Alu.mult, op1=Alu.mult)
                invprev = invp.tile([128, 1], f32, tag="inv")
                nc.vector.reciprocal(invprev[:], s_t[:])
                nc.vector.scalar_tensor_tensor(
                    B_t[:], B_t[:], r_t[:], e_t[:], op0=Alu.mult, op1=Alu.add)
                npm = erp.tile([128, FSC], f32, tag="npm")
                nc.vector.tensor_scalar(npm[:], B_t[:], 0.0, None, op0=Alu.is_le)
                amin = erp.tile([128, FSC], f32, tag="amin")
                nc.vector.scalar_tensor_tensor(
                    amin[:], npm[:], BIG, B_t[:], op0=Alu.mult, op1=Alu.add)
                mscr = scrp.tile([128, 32], f32, tag="mscr")
                nc.vector.tensor_reduce(mscr[:, 0:1], amin[:], axis=X,
                                        op=Alu.min)
                m2 = scrp.tile([128, 32], f32, tag="m2")
                nc.vector.tensor_copy(m2[:], mscr[:, 0:1].to_broadcast([128, 32]))
                nc.vector.tensor_reduce(vt8[:, 0:1], m2[:], axis=X, op=Alu.min,
                                        apply_transpose=True)
                nc.vector.match_replace(out=B_t[:], in_to_replace=vt8[:],
                                        in_values=B_t[:], imm_value=0.0)

        # ============ PHASE 3: masked softmax + @V ============
        attnT_sb = [ppool.tile([128, S], f32, tag=f"attnT{m}", name=f"attnT{m}")
                    for m in range(2)]
        with tc.tile_pool(name="p3a", bufs=2) as p3a, \
             tc.tile_pool(name="p3b", bufs=2) as p3b, \
             tc.tile_pool(name="p3sm", bufs=3) as p3sm, \
             tc.tile_pool(name="p3pt", bufs=4) as p3pt, \
             tc.tile_pool(name="p3ps", bufs=2, space="PSUM") as p3ps, \
             tc.tile_pool(name="p3ps2", bufs=2, space="PSUM") as p3ps2:
            for h in range(NH):
                for qb in range(NQ):
                    q0 = qb * 128
                    awq = p3a.tile([128, S], f32, tag="awq")
                    nc.sync.dma_start(awq[:], awS[q0:q0 + 128, h * S:(h + 1) * S])
                    eq = p3b.tile([128, S], f32, tag="eq")
                    nc.scalar.activation(eq[:], awq[:], Act.Exp)
                    mh = p3a.tile([128, S], f32, tag="mh")
                    nc.sync.dma_start(mh[:],
                                      masksS[q0:q0 + 128, h * S:(h + 1) * S])
                    mw = p3b.tile([128, S], f32, tag="mw")
                    nc.vector.tensor_tensor(mw[:], mh[:],
                                            band_sb[:, S - q0:2 * S - q0],
                                            op=Alu.max)
                    em = p3b.tile([128, S], f32, tag="em")
                    den = p3sm.tile([128, 1], f32, tag="den")
                    if emin == 0.0:
                        nc.vector.scalar_tensor_tensor(
                            em[:], eq[:], 1.0, mw[:], op0=Alu.mult,
                            op1=Alu.mult, accum_out=den[:])
                    else:
                        d_t = p3b.tile([128, S], f32, tag="dt")
                        nc.vector.tensor_scalar(d_t[:], eq[:], emin, None,
                                                op0=Alu.subtract)
                        x_t = p3b.tile([128, S], f32, tag="xt")
                        nc.vector.tensor_mul(x_t[:], d_t[:], mw[:])
                        nc.vector.tensor_scalar(em[:], x_t[:], emin, None,
                                                op0=Alu.add, accum_out=den[:])
                    rd = p3sm.tile([128, 1], f32, tag="rd")
                    nc.vector.reciprocal(rd[:], den[:])
                    dg = p3pt.tile([128, 128], f32, tag="dg")
                    nc.vector.tensor_mul(dg[:], ident_sb[:],
                                         rd[:].to_broadcast([128, 128]))
                    oT = p3ps2.tile([64, 128], f32, tag="oT")
                    for c in range(S // 128):
                        tp = p3ps.tile([128, 128], f32, tag=f"tp{c % 2}")
                        nc.tensor.matmul(tp[:], em[:, c * 128:(c + 1) * 128],
                                         dg[:], start=True, stop=True)
                        pt = p3pt.tile([128, 128], f32, tag="pt")
                        nc.scalar.copy(pt[:], tp[:])
                        nc.tensor.matmul(oT[:], v_sb[:, c, 64 * h:64 * h + 64],
                                         pt[:], start=(c == 0),
                                         stop=(c == S // 128 - 1))
                    nc.scalar.copy(
                        attnT_sb[h // 2][64 * (h % 2):64 * (h % 2) + 64,
                                         q0:q0 + 128], oT[:])

            # ---- out projection ----
            woT_sb = [p3a.tile([128, E], f32, tag=f"woT{m}", name=f"woT{m}", bufs=1)
                      for m in range(2)]
            for m in range(2):
                nc.sync.dma_start(woT_sb[m][:], woT[m * 128:(m + 1) * 128, :])
            for qb in range(NQ):
                q0 = qb * 128
                for eb in range(E // 512):
                    ops_ = p3ps2.tile([128, 512], f32, tag="wops")
                    for dc in range(2):
                        nc.tensor.matmul(ops_[:],
                                         attnT_sb[dc][:, q0:q0 + 128],
                                         woT_sb[dc][:, eb * 512:(eb + 1) * 512],
                                         start=(dc == 0), stop=(dc == 1))
                    ot = p3pt.tile([128, 512], f32, tag="wot")
                    nc.scalar.copy(ot[:], ops_[:])
                    nc.sync.dma_start(
                        partial[q0:q0 + 128, eb * 512:(eb + 1) * 512], ot[:])

    nc.compile()
    return nc


def _host_prep(core, hidden_states, attention_mask, Wq, bq, Wk, bk, Wv, bv,
               Wo, bo, with_bias):
    FSC = S // 32
    b, h0 = core // 4, (core * NH) % H
    sl = slice(h0 * D, (h0 + NH) * D)
    f32 = np.float32
    j = (np.arange(128)[:, None] % 32) * FSC + np.arange(FSC)[None, :]
    oneh = (j.reshape(-1)[None, :] == np.arange(S)[:, None]).astype(f32)
    i = np.arange(128)[:, None]
    c = np.arange(2 * S)[None, :]
    band = ((c >= S + i - RB) & (c <= S + i)).astype(f32)
    m = {
        "hsT": np.ascontiguousarray(hidden_states[b].T.astype(f32)),
        "wqT": np.ascontiguousarray(Wq[sl, :].T.astype(f32)),
        "wkT": np.ascontiguousarray(Wk[sl, :].T.astype(f32)),
        "wvT": np.ascontiguousarray(Wv[sl, :].T.astype(f32)),
        "woT": np.ascontiguousarray(Wo[:, sl].T.astype(f32)),
        "amask": np.ascontiguousarray(attention_mask[b, 0].astype(f32)),
        "oneh": oneh, "band": band,
        "ident": np.eye(128, dtype=f32),
        "idxf": j.astype(f32),
    }
    if with_bias:
        m["bq2"] = np.ascontiguousarray(
            (bq[sl] * SCALE).reshape(2, 128).T.astype(f32))
        m["bk2"] = np.ascontiguousarray(bk[sl].reshape(2, 128).T.astype(f32))
        m["bvr"] = np.tile(bv[sl].astype(f32), (128, 1))
    return m


def prepared(hidden_states, attention_mask, Wq, bq, Wk, bk, Wv, bv, Wo, bo):
    """Returns (compiled nc, per-core input maps)."""
    args = (hidden_states, attention_mask, Wq, bq, Wk, bk, Wv, bv, Wo, bo)
    args = tuple(np.asarray(a) for a in args)
    (hidden_states, attention_mask, Wq, bq, Wk, bk, Wv, bv, Wo, bo) = args
    with_bias = bool(np.any(bq) or np.any(bk) or np.any(bv))
    maskmin = float(attention_mask.min())
    key = (with_bias, maskmin)
    if key not in _CACHE:
        _CACHE[key] = _build(with_bias, maskmin)
    nc = _CACHE[key]
    in_maps = [_host_prep(core, hidden_states, attention_mask, Wq, bq, Wk, bk,
                          Wv, bv, Wo, bo, with_bias) for core in range(8)]
    return nc, in_maps


def kernel(hidden_states, attention_mask, Wq, bq, Wk, bk, Wv, bv, Wo, bo):
    from concourse.bass_utils import run_bass_kernel_spmd
    nc, in_maps = prepared(hidden_states, attention_mask, Wq, bq, Wk, bk,
                           Wv, bv, Wo, bo)
    res = run_bass_kernel_spmd(nc, in_maps, core_ids=list(range(8)))
    out = np.zeros((2, S, E), np.float32)
    for c, r in enumerate(res.results):
        out[c // 4] += r["partial"]
    out += np.asarray(bo, np.float32)[None, None, :]
    return out

